# revision 31
# baseline (speedup 1.0000x reference)
"""Trainium2 Bass kernel for nn_LocalSolverCore (sparse local-window attention solver).

Sharding: 8 cores = 2 batches x 4 sequence-chunks of 512 tokens.
Per transformer block: AllGather halo exchange of pre-LayerNormed fp16 tiles
(128 tokens each side) within each batch group of 4 cores; banded attention
computed in transposed score layout [k_part, q_free]; exp into a packed bf16
prob buffer (double-buffered by head parity) with one merged multiplicative
0/1 band-mask multiply per head on the vector engine; softmax denominator as
a 65th V row, reciprocal+partition-broadcast, divided out on the vector
engine; wo matmuls for head h-2 emitted inside head h's exp-wait window; MLP
split into tile pairs (0,3) then (1,2) so the halo AllGather launches early,
with the next block's tile-1/4 transposes and V projections emitted at the
tail; cross-attention completion deferred into the next block past the
AllReduce. LN gains/biases are folded into weight matrices host-side.
"""

import os
import numpy as np
import ml_dtypes

import concourse.bass as bass
import concourse.mybir as mybir
import concourse.tile as tile
from concourse import bacc
from concourse.bass_utils import run_bass_kernel_spmd

BF16 = mybir.dt.bfloat16
F16 = mybir.dt.float16
F32 = mybir.dt.float32
F32R = mybir.dt.float32r
I32 = mybir.dt.int32
NPBF16 = ml_dtypes.bfloat16
NPF16 = np.float16
AF = mybir.ActivationFunctionType
ALU = mybir.AluOpType

B, T, D_IN, D = 2, 2048, 4096, 512
H, DH, W_WIN, NM = 8, 64, 128, 16
K_OUTER, K_INNER = 3, 4
NH_X, DH_X = 4, 128
EPS = 1e-5
C = 512
EXT = C + 2 * W_WIN          # 768
NT_OWN, NT_EXT = 4, 6
NDC = 4                      # D/128
N_DIN = 32                   # D_IN/128
NCHUNK = 4

# k-tile j (ext rows [128j,128j+128)) -> q window [qlo, qhi)
QWIN = [(0, 128), (0, 256), (0, 384), (128, 512), (256, 512), (384, 512)]
# packed prob layout: j-window offsets within [128, 1536]
POFF = [0, 128, 384, 768, 1152, 1408]
# exp groups: (first j, [j list], total width)
PGRP = [(0, [0, 1], 384), (2, [2], 384), (3, [3], 384), (4, [4, 5], 384)]
GROUPS = [[0, 1, 2, 3], [4, 5, 6, 7]]

_CACHE = {}


def _build_program():
    nc = bacc.Bacc(None, target_bir_lowering=False)

    def inp(name, shape, dt=F32):
        return nc.dram_tensor(name, list(shape), dt, kind="ExternalInput")

    promptT_d = inp("promptT", [N_DIN, 128, C], F16)
    proj_in_d = inp("proj_in", [N_DIN, 128, D], F16)
    wq_d = inp("wq", [128, NDC, D], F16)
    wk_d = inp("wk", [128, NDC, D], F16)
    wv_d = inp("wv", [128, NDC, D], F16)
    wo_d = inp("wo", [64, H, D], F16)
    wg_d = inp("wg", [128, NDC, 2 * D], F16)
    wd_d = inp("wd", [128, 8, D], F16)
    hk_d = inp("hk_w", [128, NDC, D], F32R)
    hv_d = inp("hv_w", [128, NDC, D], F32R)
    hq_d = inp("hq_w", [128, NDC, D], F32R)
    how_d = inp("ho_w", [128, NDC, D], F32R)
    po_d = inp("proj_out", [NDC, 8, 128, D], BF16)
    bqk_d = inp("bqk", [128, NDC, 2])
    bg_d = inp("bg_t", [128, 8])
    bhq_d = inp("bhq_t", [128, NDC])
    bv_d = inp("bv_bc", [128, D])
    qkmw_d = inp("qkmw", [128, 1536], BF16)
    hidx_d = inp("halo_idx", [128, 2], I32)
    id_d = inp("identity", [128, 128], F32R)
    idb_d = inp("id_bf", [128, 128], F16)
    zh0_d = inp("h_init_bc", [NM, D], F32R)
    ong_d = inp("on_g_bc", [NM, D_IN], BF16)
    onb_d = inp("on_b_bc", [NM, D_IN], BF16)

    out_y = nc.dram_tensor("out_y", [NM, D_IN], F32, kind="ExternalOutput")

    ag_in = [nc.dram_tensor(f"ag_in{p}", [256, D], F16) for p in range(2)]
    ag_out = [nc.dram_tensor(f"ag_out{p}", [1024, D], F16) for p in range(2)]
    ar_in = nc.dram_tensor("ar_in", [NH_X * DH_X + NH_X, NM], F32)
    ar_out = nc.dram_tensor("ar_out", [NH_X * DH_X + NH_X, NM], F32)
    warm_in = nc.dram_tensor("warm_in", [1, 16], F32)
    warm_out = nc.dram_tensor("warm_out", [4, 16], F32)

    import contextlib
    with nc.allow_low_precision(reason="bf16 probs/f32r matmul operands are intentional"), \
            tile.TileContext(nc) as tc, contextlib.ExitStack() as ctx:
        singles = ctx.enter_context(tc.tile_pool(name="singles", bufs=1))
        psA = ctx.enter_context(tc.tile_pool(name="psA", bufs=4, space="PSUM"))
        psS = ctx.enter_context(tc.tile_pool(name="psS", bufs=2, space="PSUM"))
        psO = ctx.enter_context(tc.tile_pool(name="psO", bufs=2, space="PSUM"))
        work = ctx.enter_context(tc.tile_pool(name="work", bufs=1))
        htok = ctx.enter_context(tc.tile_pool(name="htok", bufs=2))
        carry = ctx.enter_context(tc.tile_pool(name="carry", bufs=4))
        small = ctx.enter_context(tc.tile_pool(name="small", bufs=2))
        stream = ctx.enter_context(tc.tile_pool(name="stream", bufs=2))

        def load(name, ap, shape, dt=F32):
            t = singles.tile(list(shape), dt, tag=name)
            nc.sync.dma_start(out=t[:], in_=ap)
            return t

        # warm up the collective path so the first real halo AllGather
        # does not pay CC/mesh initialization
        nc.gpsimd.collective_compute(
            "AllGather", ALU.bypass, ins=[warm_in[:]], outs=[warm_out[:]],
            replica_groups=GROUPS)

        wq_sb = load("wq", wq_d[:], [128, NDC, D], F16)
        wk_sb = load("wk", wk_d[:], [128, NDC, D], F16)
        wv_sb = load("wv", wv_d[:], [128, NDC, D], F16)
        wo_sb = load("wo", wo_d[:], [64, H, D], F16)
        wg_sb = load("wg", wg_d[:], [128, NDC, 2 * D], F16)
        wd_sb = load("wd", wd_d[:], [128, 8, D], F16)
        hk_sb = load("hk_w", hk_d[:], [128, NDC, D], F32R)
        hv_sb = load("hv_w", hv_d[:], [128, NDC, D], F32R)
        hq_sb = load("hq_w", hq_d[:], [128, NDC, D], F32R)
        how_sb = load("ho_w", how_d[:], [128, NDC, D], F32R)
        bqk_sb = load("bqk", bqk_d[:], [128, NDC, 2])
        bg_sb = load("bg_t", bg_d[:], [128, 8])
        bhq_sb = load("bhq_t", bhq_d[:], [128, NDC])
        bv_sb = load("bv_bc", bv_d[:], [128, D])
        qkmw_sb = load("qkmw", qkmw_d[:], [128, 1536], BF16)
        hidx_sb = load("halo_idx", hidx_d[:], [128, 2], I32)
        id_sb = load("identity", id_d[:], [128, 128], F32R)
        idb_sb = load("id_bf", idb_d[:], [128, 128], F16)
        zh_sb = load("h_init_bc", zh0_d[:], [NM, D], F32R)

        eps_sb = singles.tile([128, 1], F32, tag="eps")
        nc.vector.memset(eps_sb[:], EPS)

        e_sb = singles.tile([128, NT_OWN, D], F32, tag="e")
        x_sb = singles.tile([128, NT_OWN, D], F32R, tag="x")
        hal_sb = singles.tile([128, 2, D], F16, tag="hal")
        v_sb = singles.tile([128, NT_EXT, H, DH + 1], BF16, tag="v")
        v2_sb = singles.tile([128, NT_OWN, NH_X, DH_X + 1], BF16, tag="v2")
        nc.vector.memset(v_sb[:, :, :, DH:DH + 1], 1.0)
        pT_sb = singles.tile([128, 2, 1536], BF16, tag="pTs")
        nc.vector.memset(v2_sb[:, :, :, DH_X:DH_X + 1], 1.0)
        ych_sb = singles.tile([NM, 8, D], F32, tag="ych")

        def send_halo(parity, src0, src3):
            nc.scalar.dma_start(out=ag_in[parity][0:128, :], in_=src0)
            nc.scalar.dma_start(out=ag_in[parity][128:256, :], in_=src3)
            nc.gpsimd.collective_compute(
                "AllGather", ALU.bypass, ins=[ag_in[parity][:]],
                outs=[ag_out[parity][:]], replica_groups=GROUPS)

        def layernorm_tile(src_ap, dst_ap, np_=128):
            st = small.tile([np_, 6], F32, tag="bnst")
            mv = small.tile([np_, 2], F32, tag="bnmv")
            rs = small.tile([np_, 1], F32, tag="rstd")
            nc.vector.bn_stats(out=st[:], in_=src_ap)
            nc.vector.bn_aggr(out=mv[:], in_=st[:])
            nc.scalar.activation(out=rs[:], in_=mv[:, 1:2], func=AF.Sqrt,
                                 bias=eps_sb[0:np_, :], scale=1.0)
            nc.vector.reciprocal(out=rs[:], in_=rs[:])
            nc.vector.tensor_scalar(out=dst_ap, in0=src_ap,
                                    scalar1=mv[:, 0:1], scalar2=rs[:],
                                    op0=ALU.subtract, op1=ALU.mult)

        # ----- e = promptT.T @ proj_in (token-major); x = 1.1 e = z_L + e -----
        e_ps = [psA.tile([128, D], F32, tag="a", name=f"e_ps{i}") for i in range(NT_OWN)]
        for dc in range(N_DIN):
            pt_t = stream.tile([128, C], F16, tag="pt")
            pi_t = stream.tile([128, D], F16, tag="pi")
            nc.sync.dma_start(out=pt_t[:], in_=promptT_d[dc])
            nc.sync.dma_start(out=pi_t[:], in_=proj_in_d[dc])
            for tt in range(NT_OWN):
                nc.tensor.matmul(
                    e_ps[tt][:], pt_t[:, tt * 128:(tt + 1) * 128], pi_t[:],
                    start=(dc == 0), stop=(dc == N_DIN - 1))
        h1_carry = {}
        for tt in (0, 3, 1, 2):
            nc.scalar.copy(out=e_sb[:, tt, :], in_=e_ps[tt][:])
            nc.scalar.mul(out=x_sb[:, tt, :], in_=e_ps[tt][:], mul=1.1)
            if tt in (0, 3):
                ht = carry.tile([128, D], F16, tag="h1c")
                layernorm_tile(x_sb[:, tt, :], ht[:])
                h1_carry[tt] = ht
            if tt == 3:
                send_halo(0, h1_carry[0][:], h1_carry[3][:])

        def transpose_list(tts, get_src, dst_sb, pos=None, dt=F16,
                           id_t=None):
            """dst_sb[:, ds, tt*128+...] = src(tt)[:, ds*128+...].T via PE."""
            if pos is None:
                pos = {tt: tt for tt in tts}
            for g0 in range(0, len(tts), 4):
                grp = tts[g0:g0 + 4]
                idt = idb_sb if id_t is None else id_t
                pss = [psA.tile([128, 512], dt, tag="a", name=f"tp{i}")
                       for i in range(NDC)]
                for gi, tt in enumerate(grp):
                    src = get_src(tt)
                    for ds in range(NDC):
                        nc.tensor.transpose(
                            pss[ds][:, gi * 128:(gi + 1) * 128],
                            src[:, ds * 128:(ds + 1) * 128], idt[:])
                runs = [[0]]
                for gi in range(1, len(grp)):
                    if pos[grp[gi]] == pos[grp[gi - 1]] + 1:
                        runs[-1].append(gi)
                    else:
                        runs.append([gi])
                for ds in range(NDC):
                    for run in runs:
                        pr = pos[grp[run[0]]]
                        nc.vector.tensor_copy(
                            out=dst_sb[:, ds,
                                       pr * 128:(pr + len(run)) * 128],
                            in_=pss[ds][:, run[0] * 128:
                                        (run[0] + len(run)) * 128])

        # ================= main iteration =================
        PGRP2 = [PGRP[1], PGRP[2], PGRP[0], PGRP[3]]  # own-token j groups first
        pending_tail = [None]   # deferred cross-attn completion
        pre_state = [None]      # next block's hT if early work was emitted

        def emit_halo_recv(par):
            nc.gpsimd.indirect_dma_start(
                out=hal_sb[:, 0, :], out_offset=None, in_=ag_out[par][:],
                in_offset=bass.IndirectOffsetOnAxis(ap=hidx_sb[:, 0:1],
                                                    axis=0))
            nc.gpsimd.indirect_dma_start(
                out=hal_sb[:, 1, :], out_offset=None, in_=ag_out[par][:],
                in_offset=bass.IndirectOffsetOnAxis(ap=hidx_sb[:, 1:2],
                                                    axis=0))

        def v_tile(tt, hT_):
            ps = psA.tile([128, D], F32, tag="a")
            for dc in range(NDC):
                nc.tensor.matmul(
                    ps[:], hT_[:, dc, tt * 128:(tt + 1) * 128],
                    wv_sb[:, dc, :], start=(dc == 0),
                    stop=(dc == NDC - 1))
            nc.vector.tensor_tensor(
                out=v_sb[:, tt, :, 0:DH],
                in0=ps[:].rearrange("p (h d) -> p h d", h=H),
                in1=bv_sb[:].rearrange("p (h d) -> p h d", h=H),
                op=ALU.add)

        for s_outer in range(K_OUTER):
            for s_inner in range(K_INNER):
                git = s_outer * K_INNER + s_inner
                par = git % 2
                h1c_prev = h1_carry
                pre = pre_state[0]
                pre_state[0] = None

                h_cache = {}

                def get_h(tt):
                    if tt not in h_cache:
                        if tt == 0:
                            h_cache[tt] = hal_sb[:, 0, :]
                        elif tt == NT_EXT - 1:
                            h_cache[tt] = hal_sb[:, 1, :]
                        elif (tt - 1) in h1c_prev:
                            h_cache[tt] = h1c_prev[tt - 1]
                        else:
                            ht = htok.tile([128, D], F16, tag="htok")
                            layernorm_tile(x_sb[:, tt - 1, :], ht[:])
                            h_cache[tt] = ht
                    return h_cache[tt]

                if pre is None:
                    # halo receive (AllGather launched at previous tail);
                    # halos arrive pre-LayerNormed in fp16
                    emit_halo_recv(par)
                    hT = work.tile([128, NDC, EXT], F16, tag="bigA")
                    transpose_list([1, 2, 3, 4], get_h, hT)
                    for tt in (1, 2, 3, 4):
                        v_tile(tt, hT)
                else:
                    # tiles 1,4 + their V and the halo DMAs were emitted at
                    # the previous block's tail
                    hT = pre
                    transpose_list([2, 3], get_h, hT)
                    for tt in (2, 3):
                        v_tile(tt, hT)

                qT = work.tile([128, NDC, C], F16, tag="qT")
                for ot in range(NDC):
                    ps = psA.tile([128, C], F32, tag="a")
                    for dc in range(NDC):
                        nc.tensor.matmul(
                            ps[:], wq_sb[:, dc, ot * 128:(ot + 1) * 128],
                            hT[:, dc, 128:128 + C],
                            start=(dc == 0), stop=(dc == NDC - 1))
                    nc.scalar.activation(out=qT[:, ot, :], in_=ps[:],
                                         func=AF.Identity,
                                         bias=bqk_sb[:, ot, 0:1], scale=1.0)

                if pending_tail[0] is not None:
                    pending_tail[0]()
                    pending_tail[0] = None

                # halo tiles arrive -> finish hT, kT, v
                transpose_list([0, 5], get_h, hT)
                kT = work.tile([128, NDC, EXT], F16, tag="bigB")
                for ot in range(NDC):
                    for (a0, a1) in [(128, 640), (0, 256), (512, 768)]:
                        ps2 = psA.tile([128, 512], F32, tag="a")
                        for dc in range(NDC):
                            nc.tensor.matmul(
                                ps2[:, :a1 - a0],
                                wk_sb[:, dc, ot * 128:(ot + 1) * 128],
                                hT[:, dc, a0:a1],
                                start=(dc == 0), stop=(dc == NDC - 1))
                        nc.scalar.activation(out=kT[:, ot, a0:a1],
                                             in_=ps2[:, :a1 - a0],
                                             func=AF.Identity,
                                             bias=bqk_sb[:, ot, 1:2],
                                             scale=1.0)
                for tt in (0, 5):
                    v_tile(tt, hT)

                # banded attention in transposed layout [k_part, q_free]:
                # exp -> bf16 probs (double-buffered by head parity), one
                # merged multiplicative 0/1 band mask per head on DVE,
                # denominator as 65th V row; software pipeline:
                # scores(h) | pv(h-1) | wo(h-2).
                oT = work.tile([64, H, C], F16, tag="oT")
                wops = [psA.tile([128, D], F32, tag="a", name=f"wops{t}")
                        for t in range(NT_OWN)]

                def wo_head(h):
                    for tt in range(NT_OWN):
                        nc.tensor.matmul(
                            wops[tt][:], oT[:, h, tt * 128:(tt + 1) * 128],
                            wo_sb[:, h, :],
                            start=(h == 0), stop=(h == H - 1))

                def attn_head(h):
                    hp = (h % 2) * 64
                    hc = h // 2
                    pb = h % 2
                    for (j0, js, gw) in PGRP2:
                        s_ps = psS.tile([128, 384], F32, tag="s")
                        for j in js:
                            qlo, qhi = QWIN[j]
                            c0 = POFF[j] - POFF[j0]
                            nc.tensor.matmul(
                                s_ps[:, c0:c0 + qhi - qlo],
                                kT[hp:hp + 64, hc, j * 128:(j + 1) * 128],
                                qT[hp:hp + 64, hc, qlo:qhi],
                                start=True, stop=True)
                        nc.scalar.activation(
                            out=pT_sb[:, pb, POFF[j0]:POFF[j0] + gw],
                            in_=s_ps[:, :gw], func=AF.Exp, scale=1.0 / 8.0)
                    nc.vector.tensor_tensor(
                        out=pT_sb[:, pb, :], in0=pT_sb[:, pb, :],
                        in1=qkmw_sb[:], op=ALU.mult)
                    # wo of head h-2 fills the PE while exp/mask(h) complete
                    if h >= 2:
                        wo_head(h - 2)
                    o_ps = psO.tile([DH + 1, C], F32, tag="o")
                    for qt in range(NT_OWN):
                        for k, j in enumerate((qt, qt + 1, qt + 2)):
                            off = POFF[j] + qt * 128 - QWIN[j][0]
                            nc.tensor.matmul(
                                o_ps[:, qt * 128:(qt + 1) * 128],
                                v_sb[:, j, h, :],
                                pT_sb[:, pb, off:off + 128],
                                start=(k == 0), stop=(k == 2))
                    rb = small.tile([128, C], F32, tag="rb")
                    nc.scalar.copy(out=rb[DH:DH + 1, :],
                                   in_=o_ps[DH:DH + 1, :])
                    dsb = small.tile([1, C], F32, tag="dsb")
                    nc.sync.dma_start(out=dsb[:], in_=rb[DH:DH + 1, :])
                    rec = small.tile([1, C], F32, tag="rec")
                    nc.vector.reciprocal_approx_fast(out=rec[:], in_=dsb[:])
                    nc.gpsimd.partition_broadcast(rb[0:64, :], rec[:],
                                                  channels=64)
                    nc.vector.tensor_tensor(out=oT[:, h, :],
                                            in0=o_ps[0:DH, :],
                                            in1=rb[0:64, :], op=ALU.mult)

                for hh in range(H):
                    attn_head(hh)
                wo_head(H - 2)

                last_inner = (s_inner == K_INNER - 1)
                last_all = last_inner and (s_outer == K_OUTER - 1)

                # x += o @ wo ; h2 = LN(x); the last head's wo matmuls are
                # interleaved per tile so the PE works while each tile's
                # residual/LN chain runs on the vector engine
                h2_tiles = {}
                for tt in (0, 3, 1, 2):
                    nc.tensor.matmul(
                        wops[tt][:], oT[:, H - 1, tt * 128:(tt + 1) * 128],
                        wo_sb[:, H - 1, :], start=False, stop=True)
                    nc.vector.tensor_add(out=x_sb[:, tt, :],
                                         in0=x_sb[:, tt, :], in1=wops[tt][:])
                    ht = carry.tile([128, D], F16, tag="h2c")
                    layernorm_tile(x_sb[:, tt, :], ht[:])
                    h2_tiles[tt] = ht

                h2T = work.tile([128, NDC, C], F16, tag="bigB")
                gT = work.tile([128, 8, C], F16, tag="gTbuf")
                h1c = {}
                PAIR_POS = {0: 0, 3: 1, 1: 2, 2: 3}
                for pair in ((0, 3), (1, 2)):
                    cbase = PAIR_POS[pair[0]] * 128
                    transpose_list(list(pair),
                                   lambda tt: h2_tiles[tt][:], h2T,
                                   pos=PAIR_POS)
                    for gt in range(8):
                        ps = psA.tile([128, 256], F32, tag="a")
                        for dc in range(NDC):
                            nc.tensor.matmul(
                                ps[:], wg_sb[:, dc, gt * 128:(gt + 1) * 128],
                                h2T[:, dc, cbase:cbase + 256],
                                start=(dc == 0), stop=(dc == NDC - 1))
                        nc.scalar.activation(out=gT[:, gt, cbase:cbase + 256],
                                             in_=ps[:], func=AF.Silu,
                                             bias=bg_sb[:, gt:gt + 1],
                                             scale=1.0)
                    wdps = {tt: psA.tile([128, D], F32, tag="a",
                                         name=f"wdps{tt}") for tt in pair}
                    for gt in range(8):
                        for i, tt in enumerate(pair):
                            nc.tensor.matmul(
                                wdps[tt][:],
                                gT[:, gt, cbase + i * 128:cbase + (i + 1) * 128],
                                wd_sb[:, gt, :], start=(gt == 0),
                                stop=(gt == 7))
                    for tt in pair:
                        nc.vector.tensor_add(out=x_sb[:, tt, :],
                                             in0=x_sb[:, tt, :],
                                             in1=wdps[tt][:])
                        if not last_inner:
                            nc.gpsimd.tensor_add(out=x_sb[:, tt, :],
                                                 in0=x_sb[:, tt, :],
                                                 in1=e_sb[:, tt, :])
                            ht = carry.tile([128, D], F16, tag="h1c")
                            layernorm_tile(x_sb[:, tt, :], ht[:])
                            h1c[tt] = ht
                        elif not last_all and tt in (0, 3):
                            tmp = htok.tile([128, D], F32, tag="xe", bufs=1)
                            nc.vector.tensor_add(out=tmp[:],
                                                 in0=x_sb[:, tt, :],
                                                 in1=e_sb[:, tt, :])
                            ht = carry.tile([128, D], F16, tag="h1c")
                            layernorm_tile(tmp[:], ht[:])
                            h1c[tt] = ht
                    if pair == (0, 3) and not last_all:
                        send_halo((git + 1) % 2, h1c[0][:], h1c[3][:])
                        emit_halo_recv((git + 1) % 2)
                        hT2 = work.tile([128, NDC, EXT], F16, tag="bigA")
                        transpose_list([1, 4],
                                       lambda tt: h1c[tt - 1][:], hT2)
                        v_tile(1, hT2)
                        v_tile(4, hT2)
                        pre_state[0] = hT2
                h1_carry = h1c

            # ============ cross attention: z_H attends over z_L ============
            zlT = work.tile([128, NDC, C], F32R, tag="oTz")
            transpose_list([0, 1, 2, 3], lambda tt: x_sb[:, tt, :], zlT,
                           dt=F32R, id_t=id_sb)
            # restore x = z_L + e for the next outer's first inner iteration
            if s_outer < K_OUTER - 1:
                for tt in range(NT_OWN):
                    nc.vector.tensor_add(out=x_sb[:, tt, :],
                                         in0=x_sb[:, tt, :],
                                         in1=e_sb[:, tt, :])
                    if tt not in h1_carry:
                        ht = carry.tile([128, D], F16, tag="h1c")
                        layernorm_tile(x_sb[:, tt, :], ht[:])
                        h1_carry[tt] = ht

            hkT = work.tile([128, NDC, C], F32R, tag="qT")
            for ot in range(NDC):
                ps = psA.tile([128, C], F32, tag="a")
                for dc in range(NDC):
                    nc.tensor.matmul(
                        ps[:], hk_sb[:, dc, ot * 128:(ot + 1) * 128],
                        zlT[:, dc, :], start=(dc == 0), stop=(dc == NDC - 1))
                nc.scalar.copy(out=hkT[:, ot, :], in_=ps[:])
            for tt in range(NT_OWN):
                ps = psA.tile([128, D], F32, tag="a")
                for dc in range(NDC):
                    nc.tensor.matmul(
                        ps[:], zlT[:, dc, tt * 128:(tt + 1) * 128],
                        hv_sb[:, dc, :], start=(dc == 0), stop=(dc == NDC - 1))
                nc.scalar.copy(
                    out=v2_sb[:, tt, :, 0:DH_X],
                    in_=ps[:].rearrange("p (h d) -> p h d", h=NH_X))

            zh_ln = htok.tile([NM, D], F32R, tag="htok")
            layernorm_tile(zh_sb[:], zh_ln[:], np_=NM)
            zhT_ps = psA.tile([128, NDC * NM], F32R, tag="a")
            for dc in range(NDC):
                nc.tensor.transpose(
                    zhT_ps[:, dc * NM:(dc + 1) * NM],
                    zh_ln[:, dc * 128:(dc + 1) * 128], id_sb[0:NM, 0:NM])
            zhT = small.tile([128, NDC, NM], F32R, tag="zhT")
            nc.scalar.copy(out=zhT[:].rearrange("p a b -> p (a b)"),
                           in_=zhT_ps[:])
            hqT = small.tile([128, NH_X, NM], F32R, tag="hqT")
            for xh in range(NH_X):
                ps = psS.tile([128, NM], F32, tag="s")
                for dc in range(NDC):
                    nc.tensor.matmul(
                        ps[:], hq_sb[:, dc, xh * 128:(xh + 1) * 128],
                        zhT[:, dc, :], start=(dc == 0), stop=(dc == NDC - 1))
                nc.vector.tensor_scalar_add(out=hqT[:, xh, :], in0=ps[:],
                                            scalar1=bhq_sb[:, xh:xh + 1])

            for xh in range(NH_X):
                o_ps = psO.tile([DH_X, NM], F32, tag="o")
                d_ps = psA.tile([1, NM], F32, tag="a")
                for kt in range(NT_OWN):
                    s_ps = psS.tile([128, NM], F32, tag="s")
                    nc.tensor.matmul(
                        s_ps[:], hkT[:, xh, kt * 128:(kt + 1) * 128],
                        hqT[:, xh, :], start=True, stop=True)
                    px = small.tile([128, NM], BF16, tag="px")
                    nc.scalar.activation(out=px[:], in_=s_ps[:], func=AF.Exp,
                                         scale=float(1.0 / np.sqrt(DH_X)))
                    nc.tensor.matmul(o_ps[:], v2_sb[:, kt, xh, 0:DH_X], px[:],
                                     start=(kt == 0), stop=(kt == NT_OWN - 1))
                    nc.tensor.matmul(d_ps[:], v2_sb[:, kt, xh, DH_X:DH_X + 1],
                                     px[:], start=(kt == 0),
                                     stop=(kt == NT_OWN - 1))
                oxs = small.tile([DH_X, NM], F32, tag="oxs")
                nc.scalar.copy(out=oxs[:], in_=o_ps[:])
                dxs = small.tile([1, NM], F32, tag="dxs")
                nc.scalar.copy(out=dxs[:], in_=d_ps[:])
                nc.sync.dma_start(
                    out=ar_in[xh * DH_X:(xh + 1) * DH_X, :], in_=oxs[:])
                nc.sync.dma_start(
                    out=ar_in[NH_X * DH_X + xh:NH_X * DH_X + xh + 1, :],
                    in_=dxs[:])
            nc.gpsimd.collective_compute(
                "AllReduce", ALU.add, ins=[ar_in[:]], outs=[ar_out[:]],
                replica_groups=GROUPS)

            def xattn_tail():
                oxn = small.tile([128, NH_X, NM], F32R, tag="oxn")
                den = small.tile([1, NH_X * NM], F32, tag="den")
                for xh in range(NH_X):
                    nc.gpsimd.dma_start(
                        out=den[0:1, xh * NM:(xh + 1) * NM],
                        in_=ar_out[NH_X * DH_X + xh:NH_X * DH_X + xh + 1, :])
                nc.vector.reciprocal(out=den[:], in_=den[:])
                for xh in range(NH_X):
                    ox = small.tile([128, NM], F32, tag="ox")
                    nc.sync.dma_start(
                        out=ox[:], in_=ar_out[xh * DH_X:(xh + 1) * DH_X, :])
                    rb2 = small.tile([128, NM], F32, tag="rb2")
                    nc.gpsimd.partition_broadcast(
                        rb2[:], den[0:1, xh * NM:(xh + 1) * NM], channels=128)
                    nc.vector.tensor_tensor(out=oxn[:, xh, :], in0=ox[:],
                                            in1=rb2[:], op=ALU.mult)
                ho_ps = psO.tile([NM, D], F32, tag="o")
                for xh in range(NH_X):
                    nc.tensor.matmul(ho_ps[:], oxn[:, xh, :],
                                     how_sb[:, xh, :],
                                     start=(xh == 0), stop=(xh == NH_X - 1))
                nc.vector.tensor_add(out=zh_sb[:], in0=zh_sb[:],
                                     in1=ho_ps[:])

            if s_outer < K_OUTER - 1:
                pending_tail[0] = xattn_tail
            else:
                xattn_tail()

        # ================= output: LN(z_H @ proj_out) * g + b =================
        zhT2_ps = psA.tile([128, NDC * NM], F32R, tag="a")
        for dc in range(NDC):
            nc.tensor.transpose(zhT2_ps[:, dc * NM:(dc + 1) * NM],
                                zh_sb[:, dc * 128:(dc + 1) * 128],
                                id_sb[0:NM, 0:NM])
        zhT2 = small.tile([128, NDC, NM], BF16, tag="zhT2")
        nc.scalar.copy(out=zhT2[:].rearrange("p a b -> p (a b)"),
                       in_=zhT2_ps[:])
        sts = small.tile([NM, 8, 6], F32, tag="ysts")
        for ns in range(8):
            ps = psA.tile([NM, D], F32, tag="a")
            for dc in range(NDC):
                po_t = stream.tile([128, D], BF16, tag="pt")
                nc.sync.dma_start(out=po_t[:], in_=po_d[dc, ns])
                nc.tensor.matmul(ps[:], zhT2[:, dc, :], po_t[:],
                                 start=(dc == 0), stop=(dc == NDC - 1))
            nc.scalar.copy(out=ych_sb[:, ns, :], in_=ps[:])
            nc.vector.bn_stats(out=sts[:, ns, :], in_=ych_sb[:, ns, :])
        mv = small.tile([NM, 2], F32, tag="ymv")
        nc.vector.bn_aggr(out=mv[:], in_=sts[:])
        rs = small.tile([NM, 1], F32, tag="yrs")
        nc.scalar.activation(out=rs[:], in_=mv[:, 1:2], func=AF.Sqrt,
                             bias=eps_sb[0:NM, :], scale=1.0)
        nc.vector.reciprocal(out=rs[:], in_=rs[:])
        for ns in range(8):
            nc.vector.tensor_scalar(out=ych_sb[:, ns, :],
                                    in0=ych_sb[:, ns, :],
                                    scalar1=mv[:, 0:1], scalar2=rs[:],
                                    op0=ALU.subtract, op1=ALU.mult)
            ot = small.tile([NM, D], BF16, tag="ongc")
            nc.sync.dma_start(out=ot[:], in_=ong_d[:, ns * D:(ns + 1) * D])
            nc.vector.tensor_tensor(out=ych_sb[:, ns, :],
                                    in0=ych_sb[:, ns, :],
                                    in1=ot[:], op=ALU.mult)
            ob = small.tile([NM, D], BF16, tag="onbc")
            nc.sync.dma_start(out=ob[:], in_=onb_d[:, ns * D:(ns + 1) * D])
            nc.vector.tensor_tensor(out=ych_sb[:, ns, :],
                                    in0=ych_sb[:, ns, :],
                                    in1=ob[:], op=ALU.add)
            nc.sync.dma_start(out=out_y[:, ns * D:(ns + 1) * D],
                              in_=ych_sb[:, ns, :])

    nc.compile()
    return nc


def _prep_inputs(inputs):
    f = lambda k: np.asarray(inputs[k], dtype=np.float32)
    prompt = f("prompt_embeddings")
    proj_in_w = f("proj_in_w")
    bn_g, bn_b = f("bn_g"), f("bn_b")
    wq, wk, wv, wo = f("wq"), f("wk"), f("wv"), f("wo")
    fn_g, fn_b = f("fn_g"), f("fn_b")
    wg, wd = f("wg"), f("wd")
    h_init = f("h_init")
    hn_g, hn_b = f("hn_g"), f("hn_b")
    hq_w, hk_w, hv_w, ho_w = f("hq_w"), f("hk_w"), f("hv_w"), f("ho_w")
    proj_out_w = f("proj_out_w")
    on_g, on_b = f("on_g"), f("on_b")

    def chunk_w(w):  # [K, N] -> [128, K//128, N]
        dk, n = w.shape
        return np.ascontiguousarray(
            w.reshape(dk // 128, 128, n).transpose(1, 0, 2))

    bq, bk, bv = bn_b @ wq, bn_b @ wk, bn_b @ wv
    bg = fn_b @ wg
    bhq = hn_b @ hq_w
    shared = {
        "proj_in": np.ascontiguousarray(proj_in_w.reshape(N_DIN, 128, D)).astype(NPF16),
        "wq": chunk_w(bn_g[:, None] * wq).astype(NPF16),
        "wk": chunk_w(bn_g[:, None] * wk).astype(NPF16),
        "wv": chunk_w(bn_g[:, None] * wv).astype(NPF16),
        "wo": np.ascontiguousarray(wo.reshape(H, 64, D).transpose(1, 0, 2)).astype(NPF16),
        "wg": chunk_w(fn_g[:, None] * wg).astype(NPF16),
        "wd": chunk_w(wd).astype(NPF16),
        "hk_w": chunk_w(hk_w), "hv_w": chunk_w(hv_w),
        "hq_w": chunk_w(hn_g[:, None] * hq_w), "ho_w": chunk_w(ho_w),
        "proj_out": np.ascontiguousarray(
            proj_out_w.reshape(NDC, 128, 8, D).transpose(0, 2, 1, 3)).astype(NPBF16),
        "bqk": np.ascontiguousarray(
            np.stack([bq.reshape(NDC, 128).T, bk.reshape(NDC, 128).T],
                     axis=-1)),
        "bg_t": np.ascontiguousarray(bg.reshape(8, 128).T),
        "bhq_t": np.ascontiguousarray(bhq.reshape(NDC, 128).T),
        "bv_bc": np.ascontiguousarray(np.tile(bv[None, :], (128, 1))),
        "identity": np.eye(128, dtype=np.float32),
        "id_bf": np.eye(128, dtype=np.float32).astype(NPF16),
        "on_g_bc": np.ascontiguousarray(
            np.tile(on_g[None, :], (NM, 1)).astype(NPBF16)),
        "on_b_bc": np.ascontiguousarray(
            np.tile(on_b[None, :], (NM, 1)).astype(NPBF16)),
    }

    in_maps = []
    r = np.arange(128)
    for core in range(8):
        b, c = core // NCHUNK, core % NCHUNK
        start = c * C
        m = dict(shared)
        m["promptT"] = np.ascontiguousarray(
            prompt[b, start:start + C, :].T).reshape(N_DIN, 128, C).astype(NPF16)
        qkmw = np.zeros((128, 1536), np.float32)
        for j in range(NT_EXT):
            qlo, qhi = QWIN[j]
            kglob = start - W_WIN + j * 128 + r[:, None]
            qglob = start + qlo + np.arange(qhi - qlo)[None, :]
            ok = (np.abs(kglob - qglob) <= W_WIN) & (kglob >= 0) & (kglob < T)
            qkmw[:, POFF[j]:POFF[j] + qhi - qlo] = ok.astype(np.float32)
        m["qkmw"] = np.ascontiguousarray(qkmw.astype(NPBF16))
        li = ((c - 1) % NCHUNK) * 256 + 128 + r
        ri = ((c + 1) % NCHUNK) * 256 + r
        m["halo_idx"] = np.ascontiguousarray(
            np.stack([li, ri], axis=-1).astype(np.int32))
        m["h_init_bc"] = np.ascontiguousarray(
            np.broadcast_to(h_init[0], (NM, D)).astype(np.float32))
        in_maps.append(m)
    return in_maps


def kernel(**inputs):
    if "nc" not in _CACHE:
        _CACHE["nc"] = _build_program()
    nc = _CACHE["nc"]
    in_maps = _prep_inputs(inputs)
    trace = bool(os.environ.get("KBENCH_TRACE"))
    res = run_bass_kernel_spmd(nc, in_maps, core_ids=list(range(8)),
                               trace=trace)
    if trace and res.exec_time_ns is not None:
        print(f"HW exec time: {res.exec_time_ns} ns")
        _CACHE["exec_time_ns"] = res.exec_time_ns
        _CACHE["insts_and_trace"] = res.instructions_and_trace
    out = np.stack([res.results[0]["out_y"], res.results[4]["out_y"]], axis=0)
    return out.astype(np.float32)


# revision 32
# speedup vs baseline: 1.1823x; 1.1823x over previous
"""Trainium2 Bass kernel for nn_LocalSolverCore (sparse local-window attention solver).

Sharding: 8 cores = 2 batches x 4 sequence-chunks of 512 tokens.
Per transformer block: AllGather halo exchange of pre-LayerNormed fp16 tiles
(128 tokens each side) within each batch group of 4 cores; banded attention
computed in transposed score layout [k_part, q_free]; exp into a packed bf16
prob buffer (double-buffered by head parity) with one merged multiplicative
0/1 band-mask multiply per head on the vector engine; softmax denominator as
a 65th V row, reciprocal+partition-broadcast, divided out on the vector
engine; wo matmuls for head h-2 emitted inside head h's exp-wait window; MLP
split into tile pairs (0,3) then (1,2) so the halo AllGather launches early,
with the next block's tile-1/4 transposes and V projections emitted at the
tail; cross-attention completion deferred into the next block past the
AllReduce. LN gains/biases are folded into weight matrices host-side.
"""

import os
import numpy as np
import ml_dtypes

import concourse.bass as bass
import concourse.mybir as mybir
import concourse.tile as tile
from concourse import bacc
from concourse.bass_utils import run_bass_kernel_spmd

BF16 = mybir.dt.bfloat16
F16 = mybir.dt.float16
F32 = mybir.dt.float32
F32R = mybir.dt.float32r
I32 = mybir.dt.int32
NPBF16 = ml_dtypes.bfloat16
NPF16 = np.float16
AF = mybir.ActivationFunctionType
ALU = mybir.AluOpType

B, T, D_IN, D = 2, 2048, 4096, 512
H, DH, W_WIN, NM = 8, 64, 128, 16
K_OUTER, K_INNER = 3, 4
NH_X, DH_X = 4, 128
EPS = 1e-5
C = 512
EXT = C + 2 * W_WIN          # 768
NT_OWN, NT_EXT = 4, 6
NDC = 4                      # D/128
N_DIN = 32                   # D_IN/128
NCHUNK = 4

# k-tile j (ext rows [128j,128j+128)) -> q window [qlo, qhi)
QWIN = [(0, 128), (0, 256), (0, 384), (128, 512), (256, 512), (384, 512)]
# packed prob layout: j-window offsets within [128, 1536]
POFF = [0, 128, 384, 768, 1152, 1408]
# exp groups: (first j, [j list], total width)
PGRP = [(0, [0, 1], 384), (2, [2], 384), (3, [3], 384), (4, [4, 5], 384)]
GROUPS = [[0, 1, 2, 3], [4, 5, 6, 7]]

_CACHE = {}


def _build_program():
    nc = bacc.Bacc(None, target_bir_lowering=False)

    def inp(name, shape, dt=F32):
        return nc.dram_tensor(name, list(shape), dt, kind="ExternalInput")

    promptT_d = inp("promptT", [N_DIN, 128, C], F16)
    proj_in_d = inp("proj_in", [N_DIN, 128, D], F16)
    wq_d = inp("wq", [128, NDC, D], F16)
    wk_d = inp("wk", [128, NDC, D], F16)
    wv_d = inp("wv", [128, NDC, D], F16)
    wo_d = inp("wo", [64, H, D], F16)
    wg_d = inp("wg", [128, NDC, 2 * D], F16)
    wd_d = inp("wd", [128, 8, D], F16)
    hk_d = inp("hk_w", [128, NDC, D], F32R)
    hv_d = inp("hv_w", [128, NDC, D], F32R)
    hq_d = inp("hq_w", [128, NDC, D], F32R)
    how_d = inp("ho_w", [128, NDC, D], F32R)
    po_d = inp("proj_out", [NDC, 8, 128, D], BF16)
    bqk_d = inp("bqk", [128, NDC, 2])
    bg_d = inp("bg_t", [128, 8])
    bhq_d = inp("bhq_t", [128, NDC])
    bv_d = inp("bv_bc", [128, D])
    qkmw_d = inp("qkmw", [128, 1536], BF16)
    hidx_d = inp("halo_idx", [128, 2], I32)
    id_d = inp("identity", [128, 128], F32R)
    idb_d = inp("id_bf", [128, 128], F16)
    zh0_d = inp("h_init_bc", [NM, D], F32R)
    ong_d = inp("on_g_bc", [NM, D_IN], BF16)
    onb_d = inp("on_b_bc", [NM, D_IN], BF16)

    out_y = nc.dram_tensor("out_y", [NM, D_IN], F32, kind="ExternalOutput")

    ag_in = [nc.dram_tensor(f"ag_in{p}", [256, D], F16) for p in range(2)]
    ag_out = [nc.dram_tensor(f"ag_out{p}", [1024, D], F16) for p in range(2)]
    ar_in = nc.dram_tensor("ar_in", [NH_X * DH_X + NH_X, NM], F32)
    ar_out = nc.dram_tensor("ar_out", [NH_X * DH_X + NH_X, NM], F32)
    warm_in = nc.dram_tensor("warm_in", [1, 16], F32)
    warm_out = nc.dram_tensor("warm_out", [4, 16], F32)

    import contextlib
    with nc.allow_low_precision(reason="bf16 probs/f32r matmul operands are intentional"), \
            tile.TileContext(nc) as tc, contextlib.ExitStack() as ctx:
        singles = ctx.enter_context(tc.tile_pool(name="singles", bufs=1))
        psA = ctx.enter_context(tc.tile_pool(name="psA", bufs=4, space="PSUM"))
        psS = ctx.enter_context(tc.tile_pool(name="psS", bufs=2, space="PSUM"))
        psO = ctx.enter_context(tc.tile_pool(name="psO", bufs=2, space="PSUM"))
        work = ctx.enter_context(tc.tile_pool(name="work", bufs=1))
        htok = ctx.enter_context(tc.tile_pool(name="htok", bufs=2))
        carry = ctx.enter_context(tc.tile_pool(name="carry", bufs=4))
        small = ctx.enter_context(tc.tile_pool(name="small", bufs=2))
        stream = ctx.enter_context(tc.tile_pool(name="stream", bufs=2))

        def load(name, ap, shape, dt=F32):
            t = singles.tile(list(shape), dt, tag=name)
            nc.sync.dma_start(out=t[:], in_=ap)
            return t

        # warm up the collective path so the first real halo AllGather
        # does not pay CC/mesh initialization
        nc.gpsimd.collective_compute(
            "AllGather", ALU.bypass, ins=[warm_in[:]], outs=[warm_out[:]],
            replica_groups=GROUPS)

        wq_sb = load("wq", wq_d[:], [128, NDC, D], F16)
        wk_sb = load("wk", wk_d[:], [128, NDC, D], F16)
        wv_sb = load("wv", wv_d[:], [128, NDC, D], F16)
        wo_sb = load("wo", wo_d[:], [64, H, D], F16)
        wg_sb = load("wg", wg_d[:], [128, NDC, 2 * D], F16)
        wd_sb = load("wd", wd_d[:], [128, 8, D], F16)
        hk_sb = load("hk_w", hk_d[:], [128, NDC, D], F32R)
        hv_sb = load("hv_w", hv_d[:], [128, NDC, D], F32R)
        hq_sb = load("hq_w", hq_d[:], [128, NDC, D], F32R)
        how_sb = load("ho_w", how_d[:], [128, NDC, D], F32R)
        bqk_sb = load("bqk", bqk_d[:], [128, NDC, 2])
        bg_sb = load("bg_t", bg_d[:], [128, 8])
        bhq_sb = load("bhq_t", bhq_d[:], [128, NDC])
        bv_sb = load("bv_bc", bv_d[:], [128, D])
        qkmw_sb = load("qkmw", qkmw_d[:], [128, 1536], BF16)
        hidx_sb = load("halo_idx", hidx_d[:], [128, 2], I32)
        id_sb = load("identity", id_d[:], [128, 128], F32R)
        idb_sb = load("id_bf", idb_d[:], [128, 128], F16)
        zh_sb = load("h_init_bc", zh0_d[:], [NM, D], F32R)

        eps_sb = singles.tile([128, 1], F32, tag="eps")
        nc.vector.memset(eps_sb[:], EPS)

        e_sb = singles.tile([128, NT_OWN, D], F32, tag="e")
        x_sb = singles.tile([128, NT_OWN, D], F32R, tag="x")
        hal_sb = singles.tile([128, 2, D], F16, tag="hal")
        v_sb = singles.tile([128, NT_EXT, H, DH + 1], BF16, tag="v")
        v2_sb = singles.tile([128, NT_OWN, NH_X, DH_X + 1], BF16, tag="v2")
        nc.vector.memset(v_sb[:, :, :, DH:DH + 1], 1.0)
        pT_sb = singles.tile([128, 2, 1536], BF16, tag="pTs")
        nc.vector.memset(v2_sb[:, :, :, DH_X:DH_X + 1], 1.0)
        ych_sb = singles.tile([NM, 8, D], F32, tag="ych")

        def send_halo(parity, src0, src3):
            nc.scalar.dma_start(out=ag_in[parity][0:128, :], in_=src0)
            nc.scalar.dma_start(out=ag_in[parity][128:256, :], in_=src3)
            nc.gpsimd.collective_compute(
                "AllGather", ALU.bypass, ins=[ag_in[parity][:]],
                outs=[ag_out[parity][:]], replica_groups=GROUPS)

        def layernorm_tile(src_ap, dst_ap, np_=128):
            st = small.tile([np_, 6], F32, tag="bnst")
            mv = small.tile([np_, 2], F32, tag="bnmv")
            rs = small.tile([np_, 1], F32, tag="rstd")
            nc.vector.bn_stats(out=st[:], in_=src_ap)
            nc.vector.bn_aggr(out=mv[:], in_=st[:])
            nc.scalar.activation(out=rs[:], in_=mv[:, 1:2], func=AF.Sqrt,
                                 bias=eps_sb[0:np_, :], scale=1.0)
            nc.vector.reciprocal(out=rs[:], in_=rs[:])
            nc.vector.tensor_scalar(out=dst_ap, in0=src_ap,
                                    scalar1=mv[:, 0:1], scalar2=rs[:],
                                    op0=ALU.subtract, op1=ALU.mult)

        # ----- e = promptT.T @ proj_in (token-major); x = 1.1 e = z_L + e -----
        e_ps = [psA.tile([128, D], F32, tag="a", name=f"e_ps{i}") for i in range(NT_OWN)]
        for dc in range(N_DIN):
            pt_t = stream.tile([128, C], F16, tag="pt")
            pi_t = stream.tile([128, D], F16, tag="pi")
            nc.sync.dma_start(out=pt_t[:], in_=promptT_d[dc])
            nc.sync.dma_start(out=pi_t[:], in_=proj_in_d[dc])
            for tt in range(NT_OWN):
                nc.tensor.matmul(
                    e_ps[tt][:], pt_t[:, tt * 128:(tt + 1) * 128], pi_t[:],
                    start=(dc == 0), stop=(dc == N_DIN - 1))
        h1_carry = {}
        for tt in (0, 3, 1, 2):
            nc.scalar.copy(out=e_sb[:, tt, :], in_=e_ps[tt][:])
            nc.scalar.mul(out=x_sb[:, tt, :], in_=e_ps[tt][:], mul=1.1)
            if tt in (0, 3):
                ht = carry.tile([128, D], F16, tag="h1c")
                layernorm_tile(x_sb[:, tt, :], ht[:])
                h1_carry[tt] = ht
            if tt == 3:
                send_halo(0, h1_carry[0][:], h1_carry[3][:])

        def transpose_list(tts, get_src, dst_sb, pos=None, dt=F16,
                           id_t=None):
            """dst_sb[:, ds, tt*128+...] = src(tt)[:, ds*128+...].T via PE."""
            if pos is None:
                pos = {tt: tt for tt in tts}
            for g0 in range(0, len(tts), 4):
                grp = tts[g0:g0 + 4]
                idt = idb_sb if id_t is None else id_t
                pss = [psA.tile([128, 512], dt, tag="a", name=f"tp{i}")
                       for i in range(NDC)]
                for gi, tt in enumerate(grp):
                    src = get_src(tt)
                    for ds in range(NDC):
                        nc.tensor.transpose(
                            pss[ds][:, gi * 128:(gi + 1) * 128],
                            src[:, ds * 128:(ds + 1) * 128], idt[:])
                runs = [[0]]
                for gi in range(1, len(grp)):
                    if pos[grp[gi]] == pos[grp[gi - 1]] + 1:
                        runs[-1].append(gi)
                    else:
                        runs.append([gi])
                for ds in range(NDC):
                    for run in runs:
                        pr = pos[grp[run[0]]]
                        nc.vector.tensor_copy(
                            out=dst_sb[:, ds,
                                       pr * 128:(pr + len(run)) * 128],
                            in_=pss[ds][:, run[0] * 128:
                                        (run[0] + len(run)) * 128])

        # ================= main iteration =================
        PGRP2 = [PGRP[1], PGRP[2], PGRP[0], PGRP[3]]  # own-token j groups first
        pending_tail = [None]   # deferred cross-attn completion
        pre_state = [None]      # next block's hT if early work was emitted

        def emit_halo_recv(par):
            nc.gpsimd.indirect_dma_start(
                out=hal_sb[:, 0, :], out_offset=None, in_=ag_out[par][:],
                in_offset=bass.IndirectOffsetOnAxis(ap=hidx_sb[:, 0:1],
                                                    axis=0))
            nc.gpsimd.indirect_dma_start(
                out=hal_sb[:, 1, :], out_offset=None, in_=ag_out[par][:],
                in_offset=bass.IndirectOffsetOnAxis(ap=hidx_sb[:, 1:2],
                                                    axis=0))

        def v_tile(tt, hT_):
            ps = psA.tile([128, D], F32, tag="a")
            for dc in range(NDC):
                nc.tensor.matmul(
                    ps[:], hT_[:, dc, tt * 128:(tt + 1) * 128],
                    wv_sb[:, dc, :], start=(dc == 0),
                    stop=(dc == NDC - 1))
            nc.vector.tensor_tensor(
                out=v_sb[:, tt, :, 0:DH],
                in0=ps[:].rearrange("p (h d) -> p h d", h=H),
                in1=bv_sb[:].rearrange("p (h d) -> p h d", h=H),
                op=ALU.add)

        for s_outer in range(K_OUTER):
            for s_inner in range(K_INNER):
                git = s_outer * K_INNER + s_inner
                par = git % 2
                h1c_prev = h1_carry
                pre = pre_state[0]
                pre_state[0] = None

                h_cache = {}

                def get_h(tt):
                    if tt not in h_cache:
                        if tt == 0:
                            h_cache[tt] = hal_sb[:, 0, :]
                        elif tt == NT_EXT - 1:
                            h_cache[tt] = hal_sb[:, 1, :]
                        elif (tt - 1) in h1c_prev:
                            h_cache[tt] = h1c_prev[tt - 1]
                        else:
                            ht = htok.tile([128, D], F16, tag="htok")
                            layernorm_tile(x_sb[:, tt - 1, :], ht[:])
                            h_cache[tt] = ht
                    return h_cache[tt]

                if pre is None:
                    # halo receive (AllGather launched at previous tail);
                    # halos arrive pre-LayerNormed in fp16
                    emit_halo_recv(par)
                    hT = work.tile([128, NDC, EXT], F16, tag="bigA")
                    transpose_list([1, 2, 3, 4], get_h, hT)
                    for tt in (1, 2, 3, 4):
                        v_tile(tt, hT)
                else:
                    # tiles 1,4 + their V and the halo DMAs were emitted at
                    # the previous block's tail
                    hT = pre
                    transpose_list([2, 3], get_h, hT)
                    for tt in (2, 3):
                        v_tile(tt, hT)

                qT = work.tile([128, NDC, C], F16, tag="qT")
                for ot in range(NDC):
                    ps = psA.tile([128, C], F32, tag="a")
                    for dc in range(NDC):
                        nc.tensor.matmul(
                            ps[:], wq_sb[:, dc, ot * 128:(ot + 1) * 128],
                            hT[:, dc, 128:128 + C],
                            start=(dc == 0), stop=(dc == NDC - 1))
                    nc.scalar.activation(out=qT[:, ot, :], in_=ps[:],
                                         func=AF.Identity,
                                         bias=bqk_sb[:, ot, 0:1], scale=1.0)

                if pending_tail[0] is not None:
                    pending_tail[0]()
                    pending_tail[0] = None

                # halo tiles arrive -> finish hT, kT, v
                transpose_list([0, 5], get_h, hT)
                kT = work.tile([128, NDC, EXT], F16, tag="bigB")
                for ot in range(NDC):
                    for (a0, a1) in [(128, 640), (0, 256), (512, 768)]:
                        ps2 = psA.tile([128, 512], F32, tag="a")
                        for dc in range(NDC):
                            nc.tensor.matmul(
                                ps2[:, :a1 - a0],
                                wk_sb[:, dc, ot * 128:(ot + 1) * 128],
                                hT[:, dc, a0:a1],
                                start=(dc == 0), stop=(dc == NDC - 1))
                        nc.scalar.activation(out=kT[:, ot, a0:a1],
                                             in_=ps2[:, :a1 - a0],
                                             func=AF.Identity,
                                             bias=bqk_sb[:, ot, 1:2],
                                             scale=1.0)
                for tt in (0, 5):
                    v_tile(tt, hT)

                # banded attention in transposed layout [k_part, q_free]:
                # exp -> bf16 probs (double-buffered by head parity), one
                # merged multiplicative 0/1 band mask per head on DVE,
                # denominator as 65th V row; software pipeline:
                # scores(h) | pv(h-1) | wo(h-2).
                oT = work.tile([64, H, C], F16, tag="oT")
                wops = [psA.tile([128, D], F32, tag="a", name=f"wops{t}")
                        for t in range(NT_OWN)]

                def wo_head(h):
                    for tt in range(NT_OWN):
                        nc.tensor.matmul(
                            wops[tt][:], oT[:, h, tt * 128:(tt + 1) * 128],
                            wo_sb[:, h, :],
                            start=(h == 0), stop=(h == H - 1))

                def attn_head(h):
                    hp = (h % 2) * 64
                    hc = h // 2
                    pb = h % 2
                    for (j0, js, gw) in PGRP2:
                        s_ps = psS.tile([128, 384], F32, tag="s")
                        for j in js:
                            qlo, qhi = QWIN[j]
                            c0 = POFF[j] - POFF[j0]
                            nc.tensor.matmul(
                                s_ps[:, c0:c0 + qhi - qlo],
                                kT[hp:hp + 64, hc, j * 128:(j + 1) * 128],
                                qT[hp:hp + 64, hc, qlo:qhi],
                                start=True, stop=True)
                        nc.scalar.activation(
                            out=pT_sb[:, pb, POFF[j0]:POFF[j0] + gw],
                            in_=s_ps[:, :gw], func=AF.Exp, scale=1.0 / 8.0)
                    nc.vector.tensor_tensor(
                        out=pT_sb[:, pb, :], in0=pT_sb[:, pb, :],
                        in1=qkmw_sb[:], op=ALU.mult)
                    # wo of head h-2 fills the PE while exp/mask(h) complete
                    if h >= 2:
                        wo_head(h - 2)
                    o_ps = psO.tile([DH + 1, C], F32, tag="o")
                    for qt in range(NT_OWN):
                        for k, j in enumerate((qt, qt + 1, qt + 2)):
                            off = POFF[j] + qt * 128 - QWIN[j][0]
                            nc.tensor.matmul(
                                o_ps[:, qt * 128:(qt + 1) * 128],
                                v_sb[:, j, h, :],
                                pT_sb[:, pb, off:off + 128],
                                start=(k == 0), stop=(k == 2))
                    rb = small.tile([128, C], F32, tag="rb")
                    nc.scalar.copy(out=rb[DH:DH + 1, :],
                                   in_=o_ps[DH:DH + 1, :])
                    dsb = small.tile([1, C], F32, tag="dsb")
                    nc.sync.dma_start(out=dsb[:], in_=rb[DH:DH + 1, :])
                    rec = small.tile([1, C], F32, tag="rec")
                    nc.vector.reciprocal_approx_fast(out=rec[:], in_=dsb[:])
                    nc.gpsimd.partition_broadcast(rb[0:64, :], rec[:],
                                                  channels=64)
                    nc.vector.tensor_tensor(out=oT[:, h, :],
                                            in0=o_ps[0:DH, :],
                                            in1=rb[0:64, :], op=ALU.mult)

                for hh in range(H):
                    attn_head(hh)
                wo_head(H - 2)

                last_inner = (s_inner == K_INNER - 1)
                last_all = last_inner and (s_outer == K_OUTER - 1)

                # x += o @ wo ; h2 = LN(x); the last head's wo matmuls are
                # interleaved per tile so the PE works while each tile's
                # residual/LN chain runs on the vector engine
                h2_tiles = {}
                for tt in (0, 3, 1, 2):
                    nc.tensor.matmul(
                        wops[tt][:], oT[:, H - 1, tt * 128:(tt + 1) * 128],
                        wo_sb[:, H - 1, :], start=False, stop=True)
                    nc.vector.tensor_add(out=x_sb[:, tt, :],
                                         in0=x_sb[:, tt, :], in1=wops[tt][:])
                    ht = carry.tile([128, D], F16, tag="h2c")
                    layernorm_tile(x_sb[:, tt, :], ht[:])
                    h2_tiles[tt] = ht

                h2T = work.tile([128, NDC, C], F16, tag="bigB")
                gT = work.tile([128, 8, C], F16, tag="gTbuf")
                h1c = {}
                PAIR_POS = {0: 0, 3: 1, 1: 2, 2: 3}
                for pair in ((0, 3), (1, 2)):
                    cbase = PAIR_POS[pair[0]] * 128
                    transpose_list(list(pair),
                                   lambda tt: h2_tiles[tt][:], h2T,
                                   pos=PAIR_POS)
                    for gt in range(8):
                        ps = psA.tile([128, 256], F32, tag="a")
                        for dc in range(NDC):
                            nc.tensor.matmul(
                                ps[:], wg_sb[:, dc, gt * 128:(gt + 1) * 128],
                                h2T[:, dc, cbase:cbase + 256],
                                start=(dc == 0), stop=(dc == NDC - 1))
                        nc.scalar.activation(out=gT[:, gt, cbase:cbase + 256],
                                             in_=ps[:], func=AF.Silu,
                                             bias=bg_sb[:, gt:gt + 1],
                                             scale=1.0)
                    wdps = {tt: psA.tile([128, D], F32, tag="a",
                                         name=f"wdps{tt}") for tt in pair}
                    for gt in range(8):
                        for i, tt in enumerate(pair):
                            nc.tensor.matmul(
                                wdps[tt][:],
                                gT[:, gt, cbase + i * 128:cbase + (i + 1) * 128],
                                wd_sb[:, gt, :], start=(gt == 0),
                                stop=(gt == 7))
                    for tt in pair:
                        nc.vector.tensor_add(out=x_sb[:, tt, :],
                                             in0=x_sb[:, tt, :],
                                             in1=wdps[tt][:])
                        if not last_inner:
                            nc.vector.tensor_add(out=x_sb[:, tt, :],
                                                 in0=x_sb[:, tt, :],
                                                 in1=e_sb[:, tt, :])
                            ht = carry.tile([128, D], F16, tag="h1c")
                            layernorm_tile(x_sb[:, tt, :], ht[:])
                            h1c[tt] = ht
                        elif not last_all and tt in (0, 3):
                            tmp = htok.tile([128, D], F32, tag="xe", bufs=1)
                            nc.vector.tensor_add(out=tmp[:],
                                                 in0=x_sb[:, tt, :],
                                                 in1=e_sb[:, tt, :])
                            ht = carry.tile([128, D], F16, tag="h1c")
                            layernorm_tile(tmp[:], ht[:])
                            h1c[tt] = ht
                    if pair == (0, 3) and not last_all:
                        send_halo((git + 1) % 2, h1c[0][:], h1c[3][:])
                        emit_halo_recv((git + 1) % 2)
                        hT2 = work.tile([128, NDC, EXT], F16, tag="bigA")
                        transpose_list([1, 4],
                                       lambda tt: h1c[tt - 1][:], hT2)
                        v_tile(1, hT2)
                        v_tile(4, hT2)
                        pre_state[0] = hT2
                h1_carry = h1c

            # ============ cross attention: z_H attends over z_L ============
            zlT = work.tile([128, NDC, C], F32R, tag="oTz")
            transpose_list([0, 1, 2, 3], lambda tt: x_sb[:, tt, :], zlT,
                           dt=F32R, id_t=id_sb)
            # restore x = z_L + e for the next outer's first inner iteration
            if s_outer < K_OUTER - 1:
                for tt in range(NT_OWN):
                    nc.vector.tensor_add(out=x_sb[:, tt, :],
                                         in0=x_sb[:, tt, :],
                                         in1=e_sb[:, tt, :])
                    if tt not in h1_carry:
                        ht = carry.tile([128, D], F16, tag="h1c")
                        layernorm_tile(x_sb[:, tt, :], ht[:])
                        h1_carry[tt] = ht

            hkT = work.tile([128, NDC, C], F32R, tag="qT")
            for ot in range(NDC):
                ps = psA.tile([128, C], F32, tag="a")
                for dc in range(NDC):
                    nc.tensor.matmul(
                        ps[:], hk_sb[:, dc, ot * 128:(ot + 1) * 128],
                        zlT[:, dc, :], start=(dc == 0), stop=(dc == NDC - 1))
                nc.scalar.copy(out=hkT[:, ot, :], in_=ps[:])
            for tt in range(NT_OWN):
                ps = psA.tile([128, D], F32, tag="a")
                for dc in range(NDC):
                    nc.tensor.matmul(
                        ps[:], zlT[:, dc, tt * 128:(tt + 1) * 128],
                        hv_sb[:, dc, :], start=(dc == 0), stop=(dc == NDC - 1))
                nc.scalar.copy(
                    out=v2_sb[:, tt, :, 0:DH_X],
                    in_=ps[:].rearrange("p (h d) -> p h d", h=NH_X))

            zh_ln = htok.tile([NM, D], F32R, tag="htok")
            layernorm_tile(zh_sb[:], zh_ln[:], np_=NM)
            zhT_ps = psA.tile([128, NDC * NM], F32R, tag="a")
            for dc in range(NDC):
                nc.tensor.transpose(
                    zhT_ps[:, dc * NM:(dc + 1) * NM],
                    zh_ln[:, dc * 128:(dc + 1) * 128], id_sb[0:NM, 0:NM])
            zhT = small.tile([128, NDC, NM], F32R, tag="zhT")
            nc.scalar.copy(out=zhT[:].rearrange("p a b -> p (a b)"),
                           in_=zhT_ps[:])
            hqT = small.tile([128, NH_X, NM], F32R, tag="hqT")
            for xh in range(NH_X):
                ps = psS.tile([128, NM], F32, tag="s")
                for dc in range(NDC):
                    nc.tensor.matmul(
                        ps[:], hq_sb[:, dc, xh * 128:(xh + 1) * 128],
                        zhT[:, dc, :], start=(dc == 0), stop=(dc == NDC - 1))
                nc.vector.tensor_scalar_add(out=hqT[:, xh, :], in0=ps[:],
                                            scalar1=bhq_sb[:, xh:xh + 1])

            for xh in range(NH_X):
                o_ps = psO.tile([DH_X, NM], F32, tag="o")
                d_ps = psA.tile([1, NM], F32, tag="a")
                for kt in range(NT_OWN):
                    s_ps = psS.tile([128, NM], F32, tag="s")
                    nc.tensor.matmul(
                        s_ps[:], hkT[:, xh, kt * 128:(kt + 1) * 128],
                        hqT[:, xh, :], start=True, stop=True)
                    px = small.tile([128, NM], BF16, tag="px")
                    nc.scalar.activation(out=px[:], in_=s_ps[:], func=AF.Exp,
                                         scale=float(1.0 / np.sqrt(DH_X)))
                    nc.tensor.matmul(o_ps[:], v2_sb[:, kt, xh, 0:DH_X], px[:],
                                     start=(kt == 0), stop=(kt == NT_OWN - 1))
                    nc.tensor.matmul(d_ps[:], v2_sb[:, kt, xh, DH_X:DH_X + 1],
                                     px[:], start=(kt == 0),
                                     stop=(kt == NT_OWN - 1))
                oxs = small.tile([DH_X, NM], F32, tag="oxs")
                nc.scalar.copy(out=oxs[:], in_=o_ps[:])
                dxs = small.tile([1, NM], F32, tag="dxs")
                nc.scalar.copy(out=dxs[:], in_=d_ps[:])
                nc.sync.dma_start(
                    out=ar_in[xh * DH_X:(xh + 1) * DH_X, :], in_=oxs[:])
                nc.sync.dma_start(
                    out=ar_in[NH_X * DH_X + xh:NH_X * DH_X + xh + 1, :],
                    in_=dxs[:])
            nc.gpsimd.collective_compute(
                "AllReduce", ALU.add, ins=[ar_in[:]], outs=[ar_out[:]],
                replica_groups=GROUPS)

            def xattn_tail():
                oxn = small.tile([128, NH_X, NM], F32R, tag="oxn")
                den = small.tile([1, NH_X * NM], F32, tag="den")
                for xh in range(NH_X):
                    nc.gpsimd.dma_start(
                        out=den[0:1, xh * NM:(xh + 1) * NM],
                        in_=ar_out[NH_X * DH_X + xh:NH_X * DH_X + xh + 1, :])
                nc.vector.reciprocal(out=den[:], in_=den[:])
                for xh in range(NH_X):
                    ox = small.tile([128, NM], F32, tag="ox")
                    nc.sync.dma_start(
                        out=ox[:], in_=ar_out[xh * DH_X:(xh + 1) * DH_X, :])
                    rb2 = small.tile([128, NM], F32, tag="rb2")
                    nc.gpsimd.partition_broadcast(
                        rb2[:], den[0:1, xh * NM:(xh + 1) * NM], channels=128)
                    nc.vector.tensor_tensor(out=oxn[:, xh, :], in0=ox[:],
                                            in1=rb2[:], op=ALU.mult)
                ho_ps = psO.tile([NM, D], F32, tag="o")
                for xh in range(NH_X):
                    nc.tensor.matmul(ho_ps[:], oxn[:, xh, :],
                                     how_sb[:, xh, :],
                                     start=(xh == 0), stop=(xh == NH_X - 1))
                nc.vector.tensor_add(out=zh_sb[:], in0=zh_sb[:],
                                     in1=ho_ps[:])

            if s_outer < K_OUTER - 1:
                pending_tail[0] = xattn_tail
            else:
                xattn_tail()

        # ================= output: LN(z_H @ proj_out) * g + b =================
        zhT2_ps = psA.tile([128, NDC * NM], F32R, tag="a")
        for dc in range(NDC):
            nc.tensor.transpose(zhT2_ps[:, dc * NM:(dc + 1) * NM],
                                zh_sb[:, dc * 128:(dc + 1) * 128],
                                id_sb[0:NM, 0:NM])
        zhT2 = small.tile([128, NDC, NM], BF16, tag="zhT2")
        nc.scalar.copy(out=zhT2[:].rearrange("p a b -> p (a b)"),
                       in_=zhT2_ps[:])
        sts = small.tile([NM, 8, 6], F32, tag="ysts")
        for ns in range(8):
            ps = psA.tile([NM, D], F32, tag="a")
            for dc in range(NDC):
                po_t = stream.tile([128, D], BF16, tag="pt")
                nc.sync.dma_start(out=po_t[:], in_=po_d[dc, ns])
                nc.tensor.matmul(ps[:], zhT2[:, dc, :], po_t[:],
                                 start=(dc == 0), stop=(dc == NDC - 1))
            nc.scalar.copy(out=ych_sb[:, ns, :], in_=ps[:])
            nc.vector.bn_stats(out=sts[:, ns, :], in_=ych_sb[:, ns, :])
        mv = small.tile([NM, 2], F32, tag="ymv")
        nc.vector.bn_aggr(out=mv[:], in_=sts[:])
        rs = small.tile([NM, 1], F32, tag="yrs")
        nc.scalar.activation(out=rs[:], in_=mv[:, 1:2], func=AF.Sqrt,
                             bias=eps_sb[0:NM, :], scale=1.0)
        nc.vector.reciprocal(out=rs[:], in_=rs[:])
        for ns in range(8):
            nc.vector.tensor_scalar(out=ych_sb[:, ns, :],
                                    in0=ych_sb[:, ns, :],
                                    scalar1=mv[:, 0:1], scalar2=rs[:],
                                    op0=ALU.subtract, op1=ALU.mult)
            ot = small.tile([NM, D], BF16, tag="ongc")
            nc.sync.dma_start(out=ot[:], in_=ong_d[:, ns * D:(ns + 1) * D])
            nc.vector.tensor_tensor(out=ych_sb[:, ns, :],
                                    in0=ych_sb[:, ns, :],
                                    in1=ot[:], op=ALU.mult)
            ob = small.tile([NM, D], BF16, tag="onbc")
            nc.sync.dma_start(out=ob[:], in_=onb_d[:, ns * D:(ns + 1) * D])
            nc.vector.tensor_tensor(out=ych_sb[:, ns, :],
                                    in0=ych_sb[:, ns, :],
                                    in1=ob[:], op=ALU.add)
            nc.sync.dma_start(out=out_y[:, ns * D:(ns + 1) * D],
                              in_=ych_sb[:, ns, :])

    nc.compile()
    return nc


def _prep_inputs(inputs):
    f = lambda k: np.asarray(inputs[k], dtype=np.float32)
    prompt = f("prompt_embeddings")
    proj_in_w = f("proj_in_w")
    bn_g, bn_b = f("bn_g"), f("bn_b")
    wq, wk, wv, wo = f("wq"), f("wk"), f("wv"), f("wo")
    fn_g, fn_b = f("fn_g"), f("fn_b")
    wg, wd = f("wg"), f("wd")
    h_init = f("h_init")
    hn_g, hn_b = f("hn_g"), f("hn_b")
    hq_w, hk_w, hv_w, ho_w = f("hq_w"), f("hk_w"), f("hv_w"), f("ho_w")
    proj_out_w = f("proj_out_w")
    on_g, on_b = f("on_g"), f("on_b")

    def chunk_w(w):  # [K, N] -> [128, K//128, N]
        dk, n = w.shape
        return np.ascontiguousarray(
            w.reshape(dk // 128, 128, n).transpose(1, 0, 2))

    bq, bk, bv = bn_b @ wq, bn_b @ wk, bn_b @ wv
    bg = fn_b @ wg
    bhq = hn_b @ hq_w
    shared = {
        "proj_in": np.ascontiguousarray(proj_in_w.reshape(N_DIN, 128, D)).astype(NPF16),
        "wq": chunk_w(bn_g[:, None] * wq).astype(NPF16),
        "wk": chunk_w(bn_g[:, None] * wk).astype(NPF16),
        "wv": chunk_w(bn_g[:, None] * wv).astype(NPF16),
        "wo": np.ascontiguousarray(wo.reshape(H, 64, D).transpose(1, 0, 2)).astype(NPF16),
        "wg": chunk_w(fn_g[:, None] * wg).astype(NPF16),
        "wd": chunk_w(wd).astype(NPF16),
        "hk_w": chunk_w(hk_w), "hv_w": chunk_w(hv_w),
        "hq_w": chunk_w(hn_g[:, None] * hq_w), "ho_w": chunk_w(ho_w),
        "proj_out": np.ascontiguousarray(
            proj_out_w.reshape(NDC, 128, 8, D).transpose(0, 2, 1, 3)).astype(NPBF16),
        "bqk": np.ascontiguousarray(
            np.stack([bq.reshape(NDC, 128).T, bk.reshape(NDC, 128).T],
                     axis=-1)),
        "bg_t": np.ascontiguousarray(bg.reshape(8, 128).T),
        "bhq_t": np.ascontiguousarray(bhq.reshape(NDC, 128).T),
        "bv_bc": np.ascontiguousarray(np.tile(bv[None, :], (128, 1))),
        "identity": np.eye(128, dtype=np.float32),
        "id_bf": np.eye(128, dtype=np.float32).astype(NPF16),
        "on_g_bc": np.ascontiguousarray(
            np.tile(on_g[None, :], (NM, 1)).astype(NPBF16)),
        "on_b_bc": np.ascontiguousarray(
            np.tile(on_b[None, :], (NM, 1)).astype(NPBF16)),
    }

    in_maps = []
    r = np.arange(128)
    for core in range(8):
        b, c = core // NCHUNK, core % NCHUNK
        start = c * C
        m = dict(shared)
        m["promptT"] = np.ascontiguousarray(
            prompt[b, start:start + C, :].T).reshape(N_DIN, 128, C).astype(NPF16)
        qkmw = np.zeros((128, 1536), np.float32)
        for j in range(NT_EXT):
            qlo, qhi = QWIN[j]
            kglob = start - W_WIN + j * 128 + r[:, None]
            qglob = start + qlo + np.arange(qhi - qlo)[None, :]
            ok = (np.abs(kglob - qglob) <= W_WIN) & (kglob >= 0) & (kglob < T)
            qkmw[:, POFF[j]:POFF[j] + qhi - qlo] = ok.astype(np.float32)
        m["qkmw"] = np.ascontiguousarray(qkmw.astype(NPBF16))
        li = ((c - 1) % NCHUNK) * 256 + 128 + r
        ri = ((c + 1) % NCHUNK) * 256 + r
        m["halo_idx"] = np.ascontiguousarray(
            np.stack([li, ri], axis=-1).astype(np.int32))
        m["h_init_bc"] = np.ascontiguousarray(
            np.broadcast_to(h_init[0], (NM, D)).astype(np.float32))
        in_maps.append(m)
    return in_maps


def kernel(**inputs):
    if "nc" not in _CACHE:
        _CACHE["nc"] = _build_program()
    nc = _CACHE["nc"]
    in_maps = _prep_inputs(inputs)
    trace = bool(os.environ.get("KBENCH_TRACE"))
    res = run_bass_kernel_spmd(nc, in_maps, core_ids=list(range(8)),
                               trace=trace)
    if trace and res.exec_time_ns is not None:
        print(f"HW exec time: {res.exec_time_ns} ns")
        _CACHE["exec_time_ns"] = res.exec_time_ns
        _CACHE["insts_and_trace"] = res.instructions_and_trace
    out = np.stack([res.results[0]["out_y"], res.results[4]["out_y"]], axis=0)
    return out.astype(np.float32)
